# revision 8
# baseline (speedup 1.0000x reference)
"""Trainium2 Bass kernel for nn_ChaplotTextModule (LSTM text encoder, batch=1).

Computation: 1024-step LSTM (hidden 512) over embedded tokens, final h -> tiny
sigmoid attention head broadcast to [1, 64, 32, 32].

Strategy (single-core compute, replicated across the 8 cores; output read from
core 0 — batch=1 recurrence has no exploitable data parallelism and per-step
collectives would dominate):
  - x_gates = emb @ W_ih.T + b  precomputed for all steps on the PE
    (embeddings gathered by indirect DMA, PE-transposed).
  - recurrence: per step, gates = W_hh @ h via 16 fp32 matmuls with the tiny
    h-column stationary and W.T streaming; the 4 PSUM col-groups run
    concurrently (tile_position), each producing the N-slice
    {512 t + 128 c + 32 q + r} so a single DVE 32x32 block-transpose lands
    gates partition-major with h-index j = 128 c + p.
  - tanh computed via sigmoid: g-gate columns of W/bias are pre-scaled by 2 on
    the host, so ONE sigmoid activation covers all 4 gates;
    tanh(g) = 2*sigmoid(2g) - 1 is folded into the elementwise chain.
"""

import sys

sys.path.insert(0, "/opt/trn_rl_repo")

import ml_dtypes
import numpy as np

import concourse.bass as bass
import concourse.mybir as mybir
from concourse import bacc
from concourse.bass_utils import run_bass_kernel_spmd
from concourse.masks import make_identity
from concourse.tile import TileContext

EMB = 512
HID = 512
G4 = 4 * HID  # 2048
VOCAB = 50257
SEQ = 1024
ATTN = 64
IMG_H = 32
IMG_W = 32
N_CORES = 8

FP = mybir.dt.float32
BF = mybir.dt.bfloat16
AF = mybir.ActivationFunctionType
OP = mybir.AluOpType

def build_nc(unroll=8):
    nc = bacc.Bacc()

    toks_d = nc.dram_tensor("toks", [128, SEQ // 128], mybir.dt.int32, kind="ExternalInput")
    emb_d = nc.dram_tensor("emb", [VOCAB, EMB], FP, kind="ExternalInput")
    whhT_d = nc.dram_tensor("whhT", [HID, G4], BF, kind="ExternalInput")
    wihT_d = nc.dram_tensor("wihT", [EMB, G4], FP, kind="ExternalInput")
    bias_d = nc.dram_tensor("bias", [G4], FP, kind="ExternalInput")
    awT_d = nc.dram_tensor("awT", [HID, ATTN], BF, kind="ExternalInput")
    ab_d = nc.dram_tensor("ab", [1, ATTN], FP, kind="ExternalInput")
    out_h_d = nc.dram_tensor("out_h", [128, HID // 128], BF, kind="ExternalOutput")
    out_a_d = nc.dram_tensor("out_a", [ATTN, 1], FP, kind="ExternalOutput")

    NTCH = SEQ // 128  # 8 token chunks
    KCH = EMB // 128  # 4 contraction chunks
    HCH = HID // 128  # 4 h chunks

    with TileContext(nc) as tc:
        with (
            tc.tile_pool(name="persist", bufs=1) as pp,
            tc.tile_pool(name="weights", bufs=1) as wp,
        ):
            # ---- persistent tiles ----
            wk = [wp.tile([128, G4], BF, name=f"wk{k}", tag=f"wk{k}") for k in range(KCH)]
            xg_all = pp.tile([128, SEQ * 16], FP, name="xg_all", tag="xg_all")
            bias_sb = pp.tile([128, 16], FP, name="bias_sb", tag="bias_sb")
            awk = [pp.tile([128, ATTN], BF, name=f"awk{k}", tag=f"awk{k}") for k in range(KCH)]
            ab_sb = pp.tile([1, ATTN], FP, name="ab_sb", tag="ab_sb")
            ones_sb = pp.tile([1, 1], FP, name="ones_sb", tag="ones_sb")
            ident = pp.tile([128, 128], FP, name="ident", tag="ident")
            h_sb = pp.tile([128, HCH], BF, name="h_sb", tag="h_sb")
            c_sb = pp.tile([128, HCH], FP, name="c_sb", tag="c_sb")

            for k in range(KCH):
                nc.sync.dma_start(wk[k][:, :], whhT_d[128 * k : 128 * (k + 1), :])
                nc.sync.dma_start(awk[k][:, :], awT_d[128 * k : 128 * (k + 1), :])
            # bias[2048] viewed (t c p) -> [p, t, c]
            nc.sync.dma_start(
                bias_sb[:, :].rearrange("p (t c) -> p t c", t=4),
                bias_d.rearrange("(t c p) -> p t c", t=4, c=4),
            )
            nc.sync.dma_start(ab_sb[:, :], ab_d[:, :])
            nc.gpsimd.memset(ones_sb[:, :], 1.0)
            nc.gpsimd.memset(h_sb[:, :], 0.0)
            nc.gpsimd.memset(c_sb[:, :], 0.0)
            make_identity(nc, ident[:, :])

            # ---- precompute x_gates for all steps ----
            with (
                tc.tile_pool(name="pre_sbuf", bufs=2) as prep,
                tc.tile_pool(name="pre_wih", bufs=1) as wihp,
                tc.tile_pool(name="pre_psum", bufs=2, space="PSUM") as prepsum,
                tc.tile_pool(name="xsT_pool", bufs=1) as xstp,
            ):
                wih = [wihp.tile([128, G4], FP, name=f"wih{k}", tag=f"wih{k}") for k in range(KCH)]
                for k in range(KCH):
                    nc.sync.dma_start(wih[k][:, :], wihT_d[128 * k : 128 * (k + 1), :])

                tok_sb = prep.tile([128, NTCH], mybir.dt.int32, name="tok_sb", tag="tok_sb")
                nc.sync.dma_start(tok_sb[:, :], toks_d[:, :])

                xsT = [xstp.tile([128, SEQ], FP, name=f"xsT{e}", tag=f"xsT{e}") for e in range(KCH)]
                for i in range(NTCH):
                    xs_i = prep.tile([128, EMB], FP, name="xs", tag="xs")
                    nc.gpsimd.indirect_dma_start(
                        out=xs_i[:, :],
                        out_offset=None,
                        in_=emb_d[:, :],
                        in_offset=bass.IndirectOffsetOnAxis(
                            ap=tok_sb[:, i : i + 1], axis=0
                        ),
                    )
                    for e in range(KCH):
                        pst = prepsum.tile([128, 128], FP, name="pst", tag="pst")
                        nc.tensor.transpose(
                            out=pst[:, :],
                            in_=xs_i[:, 128 * e : 128 * (e + 1)],
                            identity=ident[:, :],
                        )
                        nc.vector.tensor_copy(
                            xsT[e][:, 128 * i : 128 * (i + 1)], pst[:, :]
                        )

                # xgT chunks: for (t, c): out[p, s] over all steps s
                xg_v = xg_all[:, :].rearrange("p (s w) -> p s w", w=16)
                for t in range(4):
                    for c in range(4):
                        d0 = 512 * t + 128 * c
                        for n in range(SEQ // 512):
                            psx = prepsum.tile([128, 512], FP, name="psx", tag="psx")
                            for k in range(KCH):
                                nc.tensor.matmul(
                                    psx[:, :],
                                    lhsT=wih[k][:, d0 : d0 + 128],
                                    rhs=xsT[k][:, 512 * n : 512 * (n + 1)],
                                    start=(k == 0),
                                    stop=(k == KCH - 1),
                                )
                            nc.vector.tensor_scalar_add(
                                xg_v[:, 512 * n : 512 * (n + 1), 4 * t + c],
                                psx[:, :],
                                bias_sb[:, 4 * t + c : 4 * t + c + 1],
                            )

            # ---- recurrence ----
            with (
                tc.tile_pool(name="rec_psum", bufs=2, space="PSUM") as rpsum,
                tc.tile_pool(name="jnk_psum", bufs=1, space="PSUM") as jpsum,
                tc.tile_pool(name="rec_sbuf", bufs=2) as rp,
            ):
                wv = [
                    wk[k][:, :].rearrange("p (t c r) -> p t c r", t=4, c=4)
                    for k in range(KCH)
                ]

                def step(xg_off):
                    psg = rpsum.tile([128, 512], FP, name="psg", tag="psg")
                    for q in range(4):
                        for k in range(KCH):
                            nc.tensor.matmul(
                                psg[32 * q : 32 * q + 1, :],
                                lhsT=h_sb[:, k : k + 1],
                                rhs=wv[k][:, :, :, 32 * q : 32 * (q + 1)],
                                start=(k == 0),
                                stop=(k == KCH - 1),
                                tile_position=(0, 32 * q),
                            )
                    T = rp.tile([128, 512], FP, name="T", tag="T")
                    nc.vector.transpose(T[:, :], psg[:, :])
                    G = rp.tile([128, 16], FP, name="G", tag="G")
                    nc.vector.tensor_tensor(
                        out=G[:, :],
                        in0=T[:, :].rearrange("p (b s) -> p b s", s=32)[:, :, 0],
                        in1=xg_all[:, bass.ds(xg_off, 16)],
                        op=OP.add,
                    )
                    S = rp.tile([128, 16], FP, name="S", tag="S")
                    nc.scalar.activation(S[:, :], G[:, :], AF.Sigmoid)
                    t1 = rp.tile([128, HCH], FP, name="t1", tag="t1")
                    # t1 = (sg - 0.5) * i ; later c = 2*t1 + f*c
                    nc.vector.scalar_tensor_tensor(
                        out=t1[:, :],
                        in0=S[:, 8:12],
                        scalar=0.5,
                        in1=S[:, 0:4],
                        op0=OP.subtract,
                        op1=OP.mult,
                    )
                    t2 = rp.tile([128, HCH], FP, name="t2", tag="t2")
                    nc.vector.tensor_tensor(
                        out=t2[:, :], in0=S[:, 4:8], in1=c_sb[:, :], op=OP.mult
                    )
                    nc.vector.scalar_tensor_tensor(
                        out=c_sb[:, :],
                        in0=t1[:, :],
                        scalar=2.0,
                        in1=t2[:, :],
                        op0=OP.mult,
                        op1=OP.add,
                    )
                    sc = rp.tile([128, HCH], FP, name="sc", tag="sc")
                    nc.scalar.activation(sc[:, :], c_sb[:, :], AF.Sigmoid, scale=2.0)
                    # h/2 = (sigmoid(2c) - 0.5) * o
                    nc.vector.scalar_tensor_tensor(
                        out=h_sb[:, :],
                        in0=sc[:, :],
                        scalar=0.5,
                        in1=S[:, 12:16],
                        op0=OP.subtract,
                        op1=OP.mult,
                    )
                    # keep-warm: junk matmuls tied to tail tiles so the PE
                    # never idles past a HAM window and stays at K=8/8
                    jp = jpsum.tile([1, 512], FP, name="jp", tag="jp")
                    nc.tensor.matmul(
                        jp[0:1, :],
                        lhsT=ident[:, 0:1],
                        rhs=T[:, 0:512],
                        start=True,
                        stop=True,
                    )
                    jp2 = jpsum.tile([1, 512], FP, name="jp2", tag="jp")
                    nc.tensor.matmul(
                        jp2[0:1, :],
                        lhsT=ident[:, 0:1],
                        rhs=S[:, 0:1].to_broadcast([128, 512]),
                        start=True,
                        stop=True,
                    )
                    jp3 = jpsum.tile([1, 256], FP, name="jp3", tag="jp")
                    nc.tensor.matmul(
                        jp3[0:1, :],
                        lhsT=ident[:, 0:1],
                        rhs=sc[:, 0:1].to_broadcast([128, 256]),
                        start=True,
                        stop=True,
                    )

                if unroll == 0:
                    for s in range(SEQ):
                        step(16 * s)
                else:
                    with tc.For_i(0, SEQ * 16, 16 * unroll) as iv:
                        for u in range(unroll):
                            step(iv + 16 * u)

            # ---- attention head ----
            with (
                tc.tile_pool(name="fin_psum", bufs=1, space="PSUM") as fpsum,
                tc.tile_pool(name="fin_sbuf", bufs=1) as fp_,
            ):
                psa = fpsum.tile([ATTN, 1], FP, name="psa", tag="psa")
                for k in range(KCH):
                    nc.tensor.matmul(
                        psa[:, :],
                        lhsT=awk[k][:, :],
                        rhs=h_sb[:, k : k + 1],
                        start=(k == 0),
                        stop=False,
                    )
                nc.tensor.matmul(
                    psa[:, :],
                    lhsT=ab_sb[0:1, :],
                    rhs=ones_sb[0:1, 0:1],
                    start=False,
                    stop=True,
                )
                a_sb = fp_.tile([ATTN, 1], FP, name="a_sb", tag="a_sb")
                nc.scalar.activation(a_sb[:, :], psa[:, :], AF.Sigmoid)
                nc.sync.dma_start(out_a_d[:, :], a_sb[:, :])
                nc.sync.dma_start(out_h_d[:, :], h_sb[:, :])

    nc.finalize()
    return nc


_NC_CACHE = {}


def _get_nc(unroll=8):
    if unroll not in _NC_CACHE:
        _NC_CACHE[unroll] = build_nc(unroll)
    return _NC_CACHE[unroll]


def prep_inputs(instructions_batch, emb_table, w_ih, w_hh, b_ih, b_hh, attn_w, attn_b):
    """Host-side layout prep (sharding/replication step)."""
    g = slice(2 * HID, 3 * HID)  # g-gate block of the 4H dim
    w_hh = np.asarray(w_hh, np.float32).copy()
    w_ih = np.asarray(w_ih, np.float32).copy()
    bias = (np.asarray(b_ih, np.float32) + np.asarray(b_hh, np.float32)).copy()
    w_hh[g, :] *= 2.0
    w_ih[g, :] *= 2.0
    bias[g] *= 2.0
    w_hh *= 2.0  # h is stored on-device as h/2 (sigmoid-only tail)

    toks = np.asarray(instructions_batch, np.int64).reshape(SEQ)
    in_map = {
        "toks": np.ascontiguousarray(
            toks.astype(np.int32).reshape(SEQ // 128, 128).T
        ),
        "emb": np.ascontiguousarray(np.asarray(emb_table, np.float32)),
        "whhT": np.ascontiguousarray(w_hh.T).astype(ml_dtypes.bfloat16),
        "wihT": np.ascontiguousarray(w_ih.T),
        "bias": bias,
        "awT": np.ascontiguousarray(2.0 * np.asarray(attn_w, np.float32).T).astype(ml_dtypes.bfloat16),
        "ab": np.ascontiguousarray(np.asarray(attn_b, np.float32).reshape(1, ATTN)),
    }
    return in_map


def kernel(instructions_batch, emb_table, w_ih, w_hh, b_ih, b_hh, attn_w, attn_b,
           _trace=False, _unroll=8):
    in_map = prep_inputs(
        instructions_batch, emb_table, w_ih, w_hh, b_ih, b_hh, attn_w, attn_b
    )
    nc = _get_nc(_unroll)
    res = run_bass_kernel_spmd(
        nc,
        [in_map for _ in range(N_CORES)],
        core_ids=list(range(N_CORES)),
        trace=_trace,
    )
    out = res.results[0]
    h = 2.0 * np.ascontiguousarray(out["out_h"].astype(np.float32).T).reshape(HID)  # h[128c+p] = out_h[p,c]
    attn = out["out_a"].reshape(ATTN)
    x_attention = np.broadcast_to(
        attn.astype(np.float32)[None, :, None, None], (1, ATTN, IMG_H, IMG_W)
    ).copy()
    x_instr_rep = h.astype(np.float32)[None, :]
    if _trace:
        return (x_attention, x_instr_rep), res
    return (x_attention, x_instr_rep)


# revision 9
# speedup vs baseline: 1.2283x; 1.2283x over previous
"""Trainium2 Bass kernel for nn_ChaplotTextModule (LSTM text encoder, batch=1).

Computation: 1024-step LSTM (hidden 512) over embedded tokens, final h -> tiny
sigmoid attention head broadcast to [1, 64, 32, 32].

Strategy (single-core compute, replicated across the 8 cores; output read from
core 0 — batch=1 recurrence has no exploitable data parallelism and per-step
collectives would dominate):
  - x_gates = emb @ W_ih.T + b  precomputed for all steps on the PE
    (embeddings gathered by indirect DMA, PE-transposed).
  - recurrence: per step, gates = W_hh @ h via 16 fp32 matmuls with the tiny
    h-column stationary and W.T streaming; the 4 PSUM col-groups run
    concurrently (tile_position), each producing the N-slice
    {512 t + 128 c + 32 q + r} so a single DVE 32x32 block-transpose lands
    gates partition-major with h-index j = 128 c + p.
  - tanh computed via sigmoid: g-gate columns of W/bias are pre-scaled by 2 on
    the host, so ONE sigmoid activation covers all 4 gates;
    tanh(g) = 2*sigmoid(2g) - 1 is folded into the elementwise chain.
"""

import sys

sys.path.insert(0, "/opt/trn_rl_repo")

import ml_dtypes
import numpy as np

import concourse.bass as bass
import concourse.mybir as mybir
from concourse import bacc
from concourse.bass_utils import run_bass_kernel_spmd
from concourse.masks import make_identity
from concourse.tile import TileContext

EMB = 512
HID = 512
G4 = 4 * HID  # 2048
VOCAB = 50257
SEQ = 1024
ATTN = 64
IMG_H = 32
IMG_W = 32
N_CORES = 8

FP = mybir.dt.float32
BF = mybir.dt.bfloat16
AF = mybir.ActivationFunctionType
OP = mybir.AluOpType

def build_nc(unroll=8):
    nc = bacc.Bacc()

    toks_d = nc.dram_tensor("toks", [128, SEQ // 128], mybir.dt.int32, kind="ExternalInput")
    emb_d = nc.dram_tensor("emb", [VOCAB, EMB], FP, kind="ExternalInput")
    whhT_d = nc.dram_tensor("whhT", [HID, G4], BF, kind="ExternalInput")
    wihT_d = nc.dram_tensor("wihT", [EMB, G4], FP, kind="ExternalInput")
    bias_d = nc.dram_tensor("bias", [G4], FP, kind="ExternalInput")
    awT_d = nc.dram_tensor("awT", [HID, ATTN], BF, kind="ExternalInput")
    ab_d = nc.dram_tensor("ab", [1, ATTN], FP, kind="ExternalInput")
    out_h_d = nc.dram_tensor("out_h", [128, HID // 128], BF, kind="ExternalOutput")
    out_a_d = nc.dram_tensor("out_a", [ATTN, 1], FP, kind="ExternalOutput")

    NTCH = SEQ // 128  # 8 token chunks
    KCH = EMB // 128  # 4 contraction chunks
    HCH = HID // 128  # 4 h chunks

    with TileContext(nc) as tc:
        with (
            tc.tile_pool(name="persist", bufs=1) as pp,
            tc.tile_pool(name="weights", bufs=1) as wp,
        ):
            # ---- persistent tiles ----
            wk = [wp.tile([128, G4], BF, name=f"wk{k}", tag=f"wk{k}") for k in range(KCH)]
            xg_all = pp.tile([128, SEQ * 16], FP, name="xg_all", tag="xg_all")
            bias_sb = pp.tile([128, 16], FP, name="bias_sb", tag="bias_sb")
            awk = [pp.tile([128, ATTN], BF, name=f"awk{k}", tag=f"awk{k}") for k in range(KCH)]
            ab_sb = pp.tile([1, ATTN], FP, name="ab_sb", tag="ab_sb")
            ones_sb = pp.tile([1, 1], FP, name="ones_sb", tag="ones_sb")
            ident = pp.tile([128, 128], FP, name="ident", tag="ident")
            h_sb = pp.tile([128, HCH], BF, name="h_sb", tag="h_sb")
            c_sb = pp.tile([128, HCH], FP, name="c_sb", tag="c_sb")

            for k in range(KCH):
                nc.sync.dma_start(wk[k][:, :], whhT_d[128 * k : 128 * (k + 1), :])
                nc.sync.dma_start(awk[k][:, :], awT_d[128 * k : 128 * (k + 1), :])
            # bias[2048] viewed (t c p) -> [p, t, c]
            nc.sync.dma_start(
                bias_sb[:, :].rearrange("p (t c) -> p t c", t=4),
                bias_d.rearrange("(t c p) -> p t c", t=4, c=4),
            )
            nc.sync.dma_start(ab_sb[:, :], ab_d[:, :])
            nc.gpsimd.memset(ones_sb[:, :], 1.0)
            nc.gpsimd.memset(h_sb[:, :], 0.0)
            nc.gpsimd.memset(c_sb[:, :], 0.0)
            make_identity(nc, ident[:, :])

            # ---- precompute x_gates for all steps ----
            with (
                tc.tile_pool(name="pre_sbuf", bufs=2) as prep,
                tc.tile_pool(name="pre_wih", bufs=1) as wihp,
                tc.tile_pool(name="pre_psum", bufs=2, space="PSUM") as prepsum,
                tc.tile_pool(name="xsT_pool", bufs=1) as xstp,
            ):
                wih = [wihp.tile([128, G4], FP, name=f"wih{k}", tag=f"wih{k}") for k in range(KCH)]
                for k in range(KCH):
                    nc.sync.dma_start(wih[k][:, :], wihT_d[128 * k : 128 * (k + 1), :])

                tok_sb = prep.tile([128, NTCH], mybir.dt.int32, name="tok_sb", tag="tok_sb")
                nc.sync.dma_start(tok_sb[:, :], toks_d[:, :])

                xsT = [xstp.tile([128, SEQ], FP, name=f"xsT{e}", tag=f"xsT{e}") for e in range(KCH)]
                for i in range(NTCH):
                    xs_i = prep.tile([128, EMB], FP, name="xs", tag="xs")
                    nc.gpsimd.indirect_dma_start(
                        out=xs_i[:, :],
                        out_offset=None,
                        in_=emb_d[:, :],
                        in_offset=bass.IndirectOffsetOnAxis(
                            ap=tok_sb[:, i : i + 1], axis=0
                        ),
                    )
                    for e in range(KCH):
                        pst = prepsum.tile([128, 128], FP, name="pst", tag="pst")
                        nc.tensor.transpose(
                            out=pst[:, :],
                            in_=xs_i[:, 128 * e : 128 * (e + 1)],
                            identity=ident[:, :],
                        )
                        nc.vector.tensor_copy(
                            xsT[e][:, 128 * i : 128 * (i + 1)], pst[:, :]
                        )

                # xgT chunks: for (t, c): out[p, s] over all steps s
                xg_v = xg_all[:, :].rearrange("p (s w) -> p s w", w=16)
                for t in range(4):
                    for c in range(4):
                        d0 = 512 * t + 128 * c
                        for n in range(SEQ // 512):
                            psx = prepsum.tile([128, 512], FP, name="psx", tag="psx")
                            for k in range(KCH):
                                nc.tensor.matmul(
                                    psx[:, :],
                                    lhsT=wih[k][:, d0 : d0 + 128],
                                    rhs=xsT[k][:, 512 * n : 512 * (n + 1)],
                                    start=(k == 0),
                                    stop=(k == KCH - 1),
                                )
                            nc.vector.tensor_scalar_add(
                                xg_v[:, 512 * n : 512 * (n + 1), 4 * t + c],
                                psx[:, :],
                                bias_sb[:, 4 * t + c : 4 * t + c + 1],
                            )

            # ---- recurrence ----
            with (
                tc.tile_pool(name="rec_psum", bufs=2, space="PSUM") as rpsum,
                tc.tile_pool(name="jnk_psum", bufs=1, space="PSUM") as jpsum,
                tc.tile_pool(name="rec_sbuf", bufs=2) as rp,
            ):
                wv = [
                    wk[k][:, :].rearrange("p (t c r) -> p t c r", t=4, c=4)
                    for k in range(KCH)
                ]

                def step(xg_off):
                    psg = rpsum.tile([128, 512], FP, name="psg", tag="psg")
                    for q in range(4):
                        for k in range(KCH):
                            nc.tensor.matmul(
                                psg[32 * q : 32 * q + 1, :],
                                lhsT=h_sb[:, k : k + 1],
                                rhs=wv[k][:, :, :, 32 * q : 32 * (q + 1)],
                                start=(k == 0),
                                stop=(k == KCH - 1),
                                tile_position=(0, 32 * q),
                            )
                    T = rp.tile([128, 512], FP, name="T", tag="T")
                    nc.vector.transpose(T[:, :], psg[:, :])
                    G = rp.tile([128, 16], FP, name="G", tag="G")
                    nc.vector.tensor_tensor(
                        out=G[:, :],
                        in0=T[:, :].rearrange("p (b s) -> p b s", s=32)[:, :, 0],
                        in1=xg_all[:, bass.ds(xg_off, 16)],
                        op=OP.add,
                    )
                    S = rp.tile([128, 16], FP, name="S", tag="S")
                    nc.scalar.activation(S[:, :], G[:, :], AF.Sigmoid)
                    t1 = rp.tile([128, HCH], FP, name="t1", tag="t1")
                    # t1 = (sg - 0.5) * i ; later c = 2*t1 + f*c
                    nc.vector.scalar_tensor_tensor(
                        out=t1[:, :],
                        in0=S[:, 8:12],
                        scalar=0.5,
                        in1=S[:, 0:4],
                        op0=OP.subtract,
                        op1=OP.mult,
                    )
                    t2 = rp.tile([128, HCH], FP, name="t2", tag="t2")
                    nc.vector.tensor_tensor(
                        out=t2[:, :], in0=S[:, 4:8], in1=c_sb[:, :], op=OP.mult
                    )
                    nc.vector.scalar_tensor_tensor(
                        out=c_sb[:, :],
                        in0=t1[:, :],
                        scalar=2.0,
                        in1=t2[:, :],
                        op0=OP.mult,
                        op1=OP.add,
                    )
                    sc = rp.tile([128, HCH], FP, name="sc", tag="sc")
                    nc.scalar.activation(sc[:, :], c_sb[:, :], AF.Sigmoid, scale=2.0)
                    # h/2 = (sigmoid(2c) - 0.5) * o
                    nc.vector.scalar_tensor_tensor(
                        out=h_sb[:, :],
                        in0=sc[:, :],
                        scalar=0.5,
                        in1=S[:, 12:16],
                        op0=OP.subtract,
                        op1=OP.mult,
                    )
                    # keep-warm: junk matmuls tied to tail tiles so the PE
                    # never idles past a HAM window and stays at K=8/8
                    jp = jpsum.tile([1, 512], FP, name="jp", tag="jp")
                    nc.tensor.matmul(
                        jp[0:1, :],
                        lhsT=ident[:, 0:1],
                        rhs=T[:, 0:512],
                        start=True,
                        stop=True,
                    )
                    jp2 = jpsum.tile([1, 256], FP, name="jp2", tag="jp")
                    nc.tensor.matmul(
                        jp2[0:1, :],
                        lhsT=ident[:, 0:1],
                        rhs=S[:, 0:1].to_broadcast([128, 256]),
                        start=True,
                        stop=True,
                    )

                if unroll == 0:
                    for s in range(SEQ):
                        step(16 * s)
                else:
                    with tc.For_i(0, SEQ * 16, 16 * unroll) as iv:
                        for u in range(unroll):
                            step(iv + 16 * u)

            # ---- attention head ----
            with (
                tc.tile_pool(name="fin_psum", bufs=1, space="PSUM") as fpsum,
                tc.tile_pool(name="fin_sbuf", bufs=1) as fp_,
            ):
                psa = fpsum.tile([ATTN, 1], FP, name="psa", tag="psa")
                for k in range(KCH):
                    nc.tensor.matmul(
                        psa[:, :],
                        lhsT=awk[k][:, :],
                        rhs=h_sb[:, k : k + 1],
                        start=(k == 0),
                        stop=False,
                    )
                nc.tensor.matmul(
                    psa[:, :],
                    lhsT=ab_sb[0:1, :],
                    rhs=ones_sb[0:1, 0:1],
                    start=False,
                    stop=True,
                )
                a_sb = fp_.tile([ATTN, 1], FP, name="a_sb", tag="a_sb")
                nc.scalar.activation(a_sb[:, :], psa[:, :], AF.Sigmoid)
                nc.sync.dma_start(out_a_d[:, :], a_sb[:, :])
                nc.sync.dma_start(out_h_d[:, :], h_sb[:, :])

    nc.finalize()
    return nc


_NC_CACHE = {}


def _get_nc(unroll=8):
    if unroll not in _NC_CACHE:
        _NC_CACHE[unroll] = build_nc(unroll)
    return _NC_CACHE[unroll]


def prep_inputs(instructions_batch, emb_table, w_ih, w_hh, b_ih, b_hh, attn_w, attn_b):
    """Host-side layout prep (sharding/replication step)."""
    g = slice(2 * HID, 3 * HID)  # g-gate block of the 4H dim
    w_hh = np.asarray(w_hh, np.float32).copy()
    w_ih = np.asarray(w_ih, np.float32).copy()
    bias = (np.asarray(b_ih, np.float32) + np.asarray(b_hh, np.float32)).copy()
    w_hh[g, :] *= 2.0
    w_ih[g, :] *= 2.0
    bias[g] *= 2.0
    w_hh *= 2.0  # h is stored on-device as h/2 (sigmoid-only tail)

    toks = np.asarray(instructions_batch, np.int64).reshape(SEQ)
    in_map = {
        "toks": np.ascontiguousarray(
            toks.astype(np.int32).reshape(SEQ // 128, 128).T
        ),
        "emb": np.ascontiguousarray(np.asarray(emb_table, np.float32)),
        "whhT": np.ascontiguousarray(w_hh.T).astype(ml_dtypes.bfloat16),
        "wihT": np.ascontiguousarray(w_ih.T),
        "bias": bias,
        "awT": np.ascontiguousarray(2.0 * np.asarray(attn_w, np.float32).T).astype(ml_dtypes.bfloat16),
        "ab": np.ascontiguousarray(np.asarray(attn_b, np.float32).reshape(1, ATTN)),
    }
    return in_map


def kernel(instructions_batch, emb_table, w_ih, w_hh, b_ih, b_hh, attn_w, attn_b,
           _trace=False, _unroll=8):
    in_map = prep_inputs(
        instructions_batch, emb_table, w_ih, w_hh, b_ih, b_hh, attn_w, attn_b
    )
    nc = _get_nc(_unroll)
    res = run_bass_kernel_spmd(
        nc,
        [in_map for _ in range(N_CORES)],
        core_ids=list(range(N_CORES)),
        trace=_trace,
    )
    out = res.results[0]
    h = 2.0 * np.ascontiguousarray(out["out_h"].astype(np.float32).T).reshape(HID)  # h[128c+p] = out_h[p,c]
    attn = out["out_a"].reshape(ATTN)
    x_attention = np.broadcast_to(
        attn.astype(np.float32)[None, :, None, None], (1, ATTN, IMG_H, IMG_W)
    ).copy()
    x_instr_rep = h.astype(np.float32)[None, :]
    if _trace:
        return (x_attention, x_instr_rep), res
    return (x_attention, x_instr_rep)


# revision 11
# speedup vs baseline: 1.2299x; 1.0013x over previous
"""Trainium2 Bass kernel for nn_ChaplotTextModule (LSTM text encoder, batch=1).

Computation: 1024-step LSTM (hidden 512) over embedded tokens, final h -> tiny
sigmoid attention head broadcast to [1, 64, 32, 32].

Strategy (single-core compute, replicated across the 8 cores; output read from
core 0 — batch=1 recurrence has no exploitable data parallelism and per-step
collectives would dominate):
  - x_gates = emb @ W_ih.T + b  precomputed for all steps on the PE
    (embeddings gathered by indirect DMA, PE-transposed).
  - recurrence: per step, gates = W_hh @ h via 16 fp32 matmuls with the tiny
    h-column stationary and W.T streaming; the 4 PSUM col-groups run
    concurrently (tile_position), each producing the N-slice
    {512 t + 128 c + 32 q + r} so a single DVE 32x32 block-transpose lands
    gates partition-major with h-index j = 128 c + p.
  - tanh computed via sigmoid: g-gate columns of W/bias are pre-scaled by 2 on
    the host, so ONE sigmoid activation covers all 4 gates;
    tanh(g) = 2*sigmoid(2g) - 1 is folded into the elementwise chain.
"""

import sys

sys.path.insert(0, "/opt/trn_rl_repo")

import ml_dtypes
import numpy as np

import concourse.bass as bass
import concourse.mybir as mybir
from concourse import bacc
from concourse.bass_utils import run_bass_kernel_spmd
from concourse.masks import make_identity
from concourse.tile import TileContext

EMB = 512
HID = 512
G4 = 4 * HID  # 2048
VOCAB = 50257
SEQ = 1024
ATTN = 64
IMG_H = 32
IMG_W = 32
N_CORES = 8

FP = mybir.dt.float32
BF = mybir.dt.bfloat16
AF = mybir.ActivationFunctionType
OP = mybir.AluOpType

def build_nc(unroll=8):
    nc = bacc.Bacc()

    toks_d = nc.dram_tensor("toks", [128, SEQ // 128], mybir.dt.int32, kind="ExternalInput")
    emb_d = nc.dram_tensor("emb", [VOCAB, EMB], FP, kind="ExternalInput")
    whhT_d = nc.dram_tensor("whhT", [HID, G4], BF, kind="ExternalInput")
    wihT_d = nc.dram_tensor("wihT", [EMB, G4], FP, kind="ExternalInput")
    bias_d = nc.dram_tensor("bias", [G4], FP, kind="ExternalInput")
    awT_d = nc.dram_tensor("awT", [HID, ATTN], BF, kind="ExternalInput")
    ab_d = nc.dram_tensor("ab", [1, ATTN], FP, kind="ExternalInput")
    out_h_d = nc.dram_tensor("out_h", [128, HID // 128], BF, kind="ExternalOutput")
    out_a_d = nc.dram_tensor("out_a", [ATTN, 1], FP, kind="ExternalOutput")

    NTCH = SEQ // 128  # 8 token chunks
    KCH = EMB // 128  # 4 contraction chunks
    HCH = HID // 128  # 4 h chunks

    with TileContext(nc) as tc:
        with (
            tc.tile_pool(name="persist", bufs=1) as pp,
            tc.tile_pool(name="weights", bufs=1) as wp,
        ):
            # ---- persistent tiles ----
            wk = [wp.tile([128, G4], BF, name=f"wk{k}", tag=f"wk{k}") for k in range(KCH)]
            xg_all = pp.tile([128, SEQ * 16], FP, name="xg_all", tag="xg_all")
            bias_sb = pp.tile([128, 16], FP, name="bias_sb", tag="bias_sb")
            awk = [pp.tile([128, ATTN], BF, name=f"awk{k}", tag=f"awk{k}") for k in range(KCH)]
            ab_sb = pp.tile([1, ATTN], FP, name="ab_sb", tag="ab_sb")
            ones_sb = pp.tile([1, 1], FP, name="ones_sb", tag="ones_sb")
            ident = pp.tile([128, 128], FP, name="ident", tag="ident")
            h_sb = pp.tile([128, HCH], BF, name="h_sb", tag="h_sb")
            c_sb = pp.tile([128, HCH], FP, name="c_sb", tag="c_sb")

            for k in range(KCH):
                nc.sync.dma_start(wk[k][:, :], whhT_d[128 * k : 128 * (k + 1), :])
                nc.sync.dma_start(awk[k][:, :], awT_d[128 * k : 128 * (k + 1), :])
            # bias[2048] viewed (t c p) -> [p, t, c]
            nc.sync.dma_start(
                bias_sb[:, :].rearrange("p (t c) -> p t c", t=4),
                bias_d.rearrange("(t c p) -> p t c", t=4, c=4),
            )
            nc.sync.dma_start(ab_sb[:, :], ab_d[:, :])
            nc.gpsimd.memset(ones_sb[:, :], 1.0)
            nc.gpsimd.memset(h_sb[:, :], 0.0)
            nc.gpsimd.memset(c_sb[:, :], 0.0)
            make_identity(nc, ident[:, :])

            # ---- precompute x_gates for all steps ----
            with (
                tc.tile_pool(name="pre_sbuf", bufs=2) as prep,
                tc.tile_pool(name="pre_wih", bufs=1) as wihp,
                tc.tile_pool(name="pre_psum", bufs=2, space="PSUM") as prepsum,
                tc.tile_pool(name="xsT_pool", bufs=1) as xstp,
            ):
                wih = [wihp.tile([128, G4], FP, name=f"wih{k}", tag=f"wih{k}") for k in range(KCH)]
                for k in range(KCH):
                    nc.sync.dma_start(wih[k][:, :], wihT_d[128 * k : 128 * (k + 1), :])

                tok_sb = prep.tile([128, NTCH], mybir.dt.int32, name="tok_sb", tag="tok_sb")
                nc.sync.dma_start(tok_sb[:, :], toks_d[:, :])

                xsT = [xstp.tile([128, SEQ], FP, name=f"xsT{e}", tag=f"xsT{e}") for e in range(KCH)]
                for i in range(NTCH):
                    xs_i = prep.tile([128, EMB], FP, name="xs", tag="xs")
                    nc.gpsimd.indirect_dma_start(
                        out=xs_i[:, :],
                        out_offset=None,
                        in_=emb_d[:, :],
                        in_offset=bass.IndirectOffsetOnAxis(
                            ap=tok_sb[:, i : i + 1], axis=0
                        ),
                    )
                    for e in range(KCH):
                        pst = prepsum.tile([128, 128], FP, name="pst", tag="pst")
                        nc.tensor.transpose(
                            out=pst[:, :],
                            in_=xs_i[:, 128 * e : 128 * (e + 1)],
                            identity=ident[:, :],
                        )
                        nc.vector.tensor_copy(
                            xsT[e][:, 128 * i : 128 * (i + 1)], pst[:, :]
                        )

                # xgT chunks: for (t, c): out[p, s] over all steps s
                xg_v = xg_all[:, :].rearrange("p (s w) -> p s w", w=16)
                for t in range(4):
                    for c in range(4):
                        d0 = 512 * t + 128 * c
                        for n in range(SEQ // 512):
                            psx = prepsum.tile([128, 512], FP, name="psx", tag="psx")
                            for k in range(KCH):
                                nc.tensor.matmul(
                                    psx[:, :],
                                    lhsT=wih[k][:, d0 : d0 + 128],
                                    rhs=xsT[k][:, 512 * n : 512 * (n + 1)],
                                    start=(k == 0),
                                    stop=(k == KCH - 1),
                                )
                            nc.vector.tensor_scalar_add(
                                xg_v[:, 512 * n : 512 * (n + 1), 4 * t + c],
                                psx[:, :],
                                bias_sb[:, 4 * t + c : 4 * t + c + 1],
                            )

            # ---- recurrence ----
            with (
                tc.tile_pool(name="rec_psum", bufs=2, space="PSUM") as rpsum,
                tc.tile_pool(name="jnk_psum", bufs=1, space="PSUM") as jpsum,
                tc.tile_pool(name="rec_sbuf", bufs=2) as rp,
            ):
                wv = [
                    wk[k][:, :].rearrange("p (t c r) -> p t c r", t=4, c=4)
                    for k in range(KCH)
                ]

                def step(xg_off):
                    psg = rpsum.tile([128, 512], FP, name="psg", tag="psg")
                    for q in range(4):
                        for k in range(KCH):
                            nc.tensor.matmul(
                                psg[32 * q : 32 * q + 1, :],
                                lhsT=h_sb[:, k : k + 1],
                                rhs=wv[k][:, :, :, 32 * q : 32 * (q + 1)],
                                start=(k == 0),
                                stop=(k == KCH - 1),
                                tile_position=(0, 32 * q),
                            )
                    T = rp.tile([128, 512], FP, name="T", tag="T")
                    nc.vector.transpose(T[:, :], psg[:, :])
                    G = rp.tile([128, 16], FP, name="G", tag="G")
                    nc.vector.tensor_tensor(
                        out=G[:, :],
                        in0=T[:, :].rearrange("p (b s) -> p b s", s=32)[:, :, 0],
                        in1=xg_all[:, bass.ds(xg_off, 16)],
                        op=OP.add,
                    )
                    S = rp.tile([128, 16], FP, name="S", tag="S")
                    nc.scalar.activation(S[:, :], G[:, :], AF.Sigmoid)
                    t1 = rp.tile([128, HCH], FP, name="t1", tag="t1")
                    # t1 = (sg - 0.5) * i ; later c = 2*t1 + f*c
                    nc.vector.scalar_tensor_tensor(
                        out=t1[:, :],
                        in0=S[:, 8:12],
                        scalar=0.5,
                        in1=S[:, 0:4],
                        op0=OP.subtract,
                        op1=OP.mult,
                    )
                    t2 = rp.tile([128, HCH], FP, name="t2", tag="t2")
                    nc.vector.tensor_tensor(
                        out=t2[:, :], in0=S[:, 4:8], in1=c_sb[:, :], op=OP.mult
                    )
                    nc.vector.scalar_tensor_tensor(
                        out=c_sb[:, :],
                        in0=t1[:, :],
                        scalar=2.0,
                        in1=t2[:, :],
                        op0=OP.mult,
                        op1=OP.add,
                    )
                    sc = rp.tile([128, HCH], FP, name="sc", tag="sc")
                    nc.scalar.activation(sc[:, :], c_sb[:, :], AF.Sigmoid, scale=2.0)
                    # h/2 = (sigmoid(2c) - 0.5) * o
                    nc.vector.scalar_tensor_tensor(
                        out=h_sb[:, :],
                        in0=sc[:, :],
                        scalar=0.5,
                        in1=S[:, 12:16],
                        op0=OP.subtract,
                        op1=OP.mult,
                    )
                    # keep-warm: junk matmuls tied to tail tiles so the PE
                    # never idles past a HAM window and stays at K=8/8
                    jp = jpsum.tile([1, 512], FP, name="jp", tag="jp")
                    nc.tensor.matmul(
                        jp[0:1, :],
                        lhsT=ident[:, 0:1],
                        rhs=T[:, 0:512],
                        start=True,
                        stop=True,
                    )
                    jp2 = jpsum.tile([1, 256], FP, name="jp2", tag="jp")
                    nc.tensor.matmul(
                        jp2[0:1, :],
                        lhsT=ident[:, 0:1],
                        rhs=S[:, 0:1].to_broadcast([128, 256]),
                        start=True,
                        stop=True,
                    )

                if unroll == 0:
                    for s in range(SEQ):
                        step(16 * s)
                else:
                    with tc.For_i(
                        0,
                        SEQ * 16,
                        16 * unroll,
                        hint_engines=(
                            mybir.EngineType.PE,
                            mybir.EngineType.DVE,
                            mybir.EngineType.Activation,
                        ),
                    ) as iv:
                        for u in range(unroll):
                            step(iv + 16 * u)

            # ---- attention head ----
            with (
                tc.tile_pool(name="fin_psum", bufs=1, space="PSUM") as fpsum,
                tc.tile_pool(name="fin_sbuf", bufs=1) as fp_,
            ):
                psa = fpsum.tile([ATTN, 1], FP, name="psa", tag="psa")
                for k in range(KCH):
                    nc.tensor.matmul(
                        psa[:, :],
                        lhsT=awk[k][:, :],
                        rhs=h_sb[:, k : k + 1],
                        start=(k == 0),
                        stop=False,
                    )
                nc.tensor.matmul(
                    psa[:, :],
                    lhsT=ab_sb[0:1, :],
                    rhs=ones_sb[0:1, 0:1],
                    start=False,
                    stop=True,
                )
                a_sb = fp_.tile([ATTN, 1], FP, name="a_sb", tag="a_sb")
                nc.scalar.activation(a_sb[:, :], psa[:, :], AF.Sigmoid)
                nc.sync.dma_start(out_a_d[:, :], a_sb[:, :])
                nc.sync.dma_start(out_h_d[:, :], h_sb[:, :])

    nc.finalize()
    return nc


_NC_CACHE = {}


def _get_nc(unroll=8):
    if unroll not in _NC_CACHE:
        _NC_CACHE[unroll] = build_nc(unroll)
    return _NC_CACHE[unroll]


def prep_inputs(instructions_batch, emb_table, w_ih, w_hh, b_ih, b_hh, attn_w, attn_b):
    """Host-side layout prep (sharding/replication step)."""
    g = slice(2 * HID, 3 * HID)  # g-gate block of the 4H dim
    w_hh = np.asarray(w_hh, np.float32).copy()
    w_ih = np.asarray(w_ih, np.float32).copy()
    bias = (np.asarray(b_ih, np.float32) + np.asarray(b_hh, np.float32)).copy()
    w_hh[g, :] *= 2.0
    w_ih[g, :] *= 2.0
    bias[g] *= 2.0
    w_hh *= 2.0  # h is stored on-device as h/2 (sigmoid-only tail)

    toks = np.asarray(instructions_batch, np.int64).reshape(SEQ)
    in_map = {
        "toks": np.ascontiguousarray(
            toks.astype(np.int32).reshape(SEQ // 128, 128).T
        ),
        "emb": np.ascontiguousarray(np.asarray(emb_table, np.float32)),
        "whhT": np.ascontiguousarray(w_hh.T).astype(ml_dtypes.bfloat16),
        "wihT": np.ascontiguousarray(w_ih.T),
        "bias": bias,
        "awT": np.ascontiguousarray(2.0 * np.asarray(attn_w, np.float32).T).astype(ml_dtypes.bfloat16),
        "ab": np.ascontiguousarray(np.asarray(attn_b, np.float32).reshape(1, ATTN)),
    }
    return in_map


def kernel(instructions_batch, emb_table, w_ih, w_hh, b_ih, b_hh, attn_w, attn_b,
           _trace=False, _unroll=8):
    in_map = prep_inputs(
        instructions_batch, emb_table, w_ih, w_hh, b_ih, b_hh, attn_w, attn_b
    )
    nc = _get_nc(_unroll)
    res = run_bass_kernel_spmd(
        nc,
        [in_map for _ in range(N_CORES)],
        core_ids=list(range(N_CORES)),
        trace=_trace,
    )
    out = res.results[0]
    h = 2.0 * np.ascontiguousarray(out["out_h"].astype(np.float32).T).reshape(HID)  # h[128c+p] = out_h[p,c]
    attn = out["out_a"].reshape(ATTN)
    x_attention = np.broadcast_to(
        attn.astype(np.float32)[None, :, None, None], (1, ATTN, IMG_H, IMG_W)
    ).copy()
    x_instr_rep = h.astype(np.float32)[None, :]
    if _trace:
        return (x_attention, x_instr_rep), res
    return (x_attention, x_instr_rep)
